# revision 1
# baseline (speedup 1.0000x reference)
"""Trainium2 Bass kernel for nn_NequIPNBodyNetSimple.

Reference computation (per batch b, N=256 particles, H=128 hidden, L=4 layers):
    pos = state[..., :2];  h = state @ embed_w + embed_b
    dist2[i,j] = |pos_i - pos_j|^2
    for l:  m_ij = silu([h_i, h_j, dist2_ij] @ w1 + b1) @ w2 + b2
            agg_i = sum_{j != i} m_ij
            h += concat([h, agg]) @ wu + bu
    out = state + h @ out_w + out_b

Algebraic restructure (exact up to fp reassociation):
    pre-silu a_ij = U'_i + V'_j - 2<p_i, p_j> * r + b1
      with U' = h @ W1a + |p|^2 r,  V' = h @ W1b + |p|^2 r,
           W1a = w1[:H], W1b = w1[H:2H], r = w1[2H]
    agg_i = w2.T @ (sum_{j != i} silu(a_ij)) + (N-1) b2      (linearity of sum)

Sharding: data-parallel over batch B=8 across the 8 NeuronCores (one batch
element per core); parameters replicated.

Performance structure:
  - PE emits the pairwise pre-silu tensor in [128h x 512] chunks as two
    accumulated float32r matmuls (4x the fp32 rate at >=256 moving columns).
  - ScalarE applies silu (bias=b1) from PSUM into bf16 SBUF tiles — the
    hard throughput floor (1 col/cycle regardless of dtype).
  - DVE does the j-reduction as a bf16 add-tree (2x mode) + final fp32
    reduce, plus all bias adds (fused scalar_tensor_tensor ops).
  - The layer update (diag-subtract, w2-agg, node update) and the next
    layer's U'/V'/UX prep run PER NODE-HALF, software-pipelined into the
    i-block stream so layer boundaries don't drain the engines.
"""

import numpy as np

B, N, C, H, L = 8, 256, 2, 128, 4
NCORES = 8
SI = 16   # i's per i-block
SJ = 32   # j's per j-block
NIB = N // SI   # 16 i-blocks
NJB = N // SJ   # 8 j-blocks

_PROG = {}


def _build_bass(act_name="Silu", dbg=False):
    import concourse.bass as bass
    import concourse.mybir as mybir
    import concourse.tile as tile
    from concourse import bacc
    from contextlib import ExitStack

    f32 = mybir.dt.float32
    f32r = mybir.dt.float32r
    bf16 = mybir.dt.bfloat16
    AF = mybir.ActivationFunctionType
    ALU = mybir.AluOpType
    AX = mybir.AxisListType
    SILU = getattr(AF, act_name)

    nc = bacc.Bacc()

    # ---- DRAM I/O (names must match setup_inputs keys) ----
    state = nc.dram_tensor("state", [N, 2 * C], f32, kind="ExternalInput")
    embed_w = nc.dram_tensor("embed_w", [2 * C, H], f32, kind="ExternalInput")
    embed_b = nc.dram_tensor("embed_b", [H], f32, kind="ExternalInput")
    w1 = nc.dram_tensor("w1", [L, 2 * H + 1, H], f32, kind="ExternalInput")
    b1 = nc.dram_tensor("b1", [L, H], f32, kind="ExternalInput")
    w2 = nc.dram_tensor("w2", [L, H, H], f32, kind="ExternalInput")
    b2 = nc.dram_tensor("b2", [L, H], f32, kind="ExternalInput")
    wu = nc.dram_tensor("wu", [L, 2 * H, H], f32, kind="ExternalInput")
    bu = nc.dram_tensor("bu", [L, H], f32, kind="ExternalInput")
    out_w = nc.dram_tensor("out_w", [H, 2 * C], f32, kind="ExternalInput")
    out_b = nc.dram_tensor("out_b", [2 * C], f32, kind="ExternalInput")
    out = nc.dram_tensor("out", [N, 2 * C], f32, kind="ExternalOutput")
    if dbg:
        dbg_t = {
            nm: nc.dram_tensor("dbg_" + nm, shp, f32, kind="ExternalOutput")
            for nm, shp in [
                ("uxr", [48, 8 * 512]), ("dv", [128, 512]), ("ux", [48, 16 * 128]),
                ("up", [H, N]), ("vp", [H, N]), ("s", [H, N]), ("hT0", [H, N]),
                ("rb32", [32, H]), ("posP", [32, 16]),
            ]
        }

    with tile.TileContext(nc) as tc, ExitStack() as ctx:
        const = ctx.enter_context(tc.tile_pool(name="const", bufs=1))
        wpool = ctx.enter_context(tc.tile_pool(name="wpool", bufs=2))
        work = ctx.enter_context(tc.tile_pool(name="work", bufs=2))
        silup = ctx.enter_context(tc.tile_pool(name="silup", bufs=7))
        scratch = ctx.enter_context(tc.tile_pool(name="scratch", bufs=2))
        apool = ctx.enter_context(tc.tile_pool(name="apool", bufs=3, space="PSUM"))
        mpool = ctx.enter_context(tc.tile_pool(name="mpool", bufs=2, space="PSUM"))

        # ================= one-time setup =================
        # ones132: broadcasts a [1, H] row onto 32 partitions via matmul
        ones132 = const.tile([1, 2 * SI], f32)
        nc.vector.memset(ones132, 1.0)
        # PE p-state warm-up: keep the tensor engine busy while the input
        # DMAs land so the first real matmuls run at full clock
        warm = const.tile([1, N], f32)
        nc.vector.memset(warm, 0.0)
        warm_ps = apool.tile([H, 2 * SI * SJ], f32, tag="apsum")
        for w in range(2):
            nc.tensor.matmul(
                out=warm_ps[0 : 2 * SI, 0:N],
                lhsT=ones132,
                rhs=warm,
                start=True,
                stop=True,
            )

        def load_weights(l):
            t = {}
            t["w1a"] = wpool.tile([H, H], f32, name="w1a", tag="w1a")
            nc.sync.dma_start(out=t["w1a"], in_=w1[l, 0:H, :])
            t["w1b"] = wpool.tile([H, H], f32, name="w1b", tag="w1b")
            nc.sync.dma_start(out=t["w1b"], in_=w1[l, H : 2 * H, :])
            t["r_sb"] = wpool.tile([1, H], f32, name="r_sb", tag="r_sb")
            nc.sync.dma_start(out=t["r_sb"], in_=w1[l, 2 * H : 2 * H + 1, :])
            t["b1_sb"] = wpool.tile([H, 1], f32, name="b1_sb", tag="b1_sb")
            nc.sync.dma_start(out=t["b1_sb"], in_=b1[l, :].rearrange("(h x) -> h x", x=1))
            t["w2_sb"] = wpool.tile([H, H], f32, name="w2_sb", tag="w2_sb")
            nc.sync.dma_start(out=t["w2_sb"], in_=w2[l, :, :])
            t["wu_t"] = wpool.tile([H, H], f32, name="wu_t", tag="wu_t")
            nc.sync.dma_start(out=t["wu_t"], in_=wu[l, 0:H, :])
            t["wu_b"] = wpool.tile([H, H], f32, name="wu_b", tag="wu_b")
            nc.sync.dma_start(out=t["wu_b"], in_=wu[l, H : 2 * H, :])
            t["b2_sb"] = wpool.tile([H, 1], f32, name="b2_sb", tag="b2_sb")
            nc.sync.dma_start(out=t["b2_sb"], in_=b2[l, :].rearrange("(h x) -> h x", x=1))
            t["bu_sb"] = wpool.tile([H, 1], f32, name="bu_sb", tag="bu_sb")
            nc.sync.dma_start(out=t["bu_sb"], in_=bu[l, :].rearrange("(h x) -> h x", x=1))
            t["b2x"] = wpool.tile([H, 1], f32, name="b2x", tag="b2x")
            nc.vector.tensor_scalar_mul(t["b2x"], t["b2_sb"], float(N - 1))
            t["rneg2"] = wpool.tile([1, H], f32, name="rneg2", tag="rneg2")
            nc.vector.tensor_scalar_mul(t["rneg2"], t["r_sb"], -2.0)
            # rneg2b32[p, h] = -2*r[h] on 32 partitions (for the UX pos rows)
            r32_ps = mpool.tile([H, N], f32, tag="mps")
            nc.tensor.matmul(
                out=r32_ps[0 : 2 * SI, 0:H], lhsT=ones132, rhs=t["rneg2"],
                start=True, stop=True,
            )
            t["rneg2b32"] = wpool.tile([2 * SI, H], f32, name="rneg2b32", tag="rneg2b32")
            nc.vector.tensor_copy(out=t["rneg2b32"], in_=r32_ps[0 : 2 * SI, 0:H])
            return t

        # stateT[c, i] = state[i, c]   (4 x 256)
        stateT = const.tile([2 * C, N], f32)
        nc.sync.dma_start(out=stateT, in_=state[:, :].rearrange("n c -> c n"))
        posT = stateT[0:2, :]  # [2, 256]

        # sq_flat[0, i] = |pos_i|^2  via ones-matmul partition reduction
        possq = const.tile([2, N], f32)
        nc.vector.tensor_mul(possq, posT, posT)
        ones2 = const.tile([2, 1], f32)
        nc.vector.memset(ones2, 1.0)
        sq_ps = mpool.tile([H, N], f32, tag="mps")
        nc.tensor.matmul(out=sq_ps[0:1, :], lhsT=ones2, rhs=possq, start=True, stop=True)
        sq_flat = const.tile([1, N], f32)
        nc.vector.tensor_copy(out=sq_flat, in_=sq_ps[0:1, :])

        # ---- embedding: hT = (state @ embed_w + embed_b)^T ----
        embw_sb = const.tile([2 * C, H], f32)
        nc.sync.dma_start(out=embw_sb, in_=embed_w[:, :])
        embb_sb = const.tile([H, 1], f32)
        nc.sync.dma_start(out=embb_sb, in_=embed_b[:].rearrange("(h x) -> h x", x=1))
        h_ps = mpool.tile([H, N], f32, tag="mps")
        nc.tensor.matmul(out=h_ps, lhsT=embw_sb, rhs=stateT, start=True, stop=True)
        hT = work.tile([H, N], f32, tag="hT")
        nc.vector.tensor_scalar_add(hT, h_ps, embb_sb)

        W = [None] * L
        W[0] = load_weights(0)

        # posP[16*c+ii, ib] = pos[SI*ib+ii, c]  (stationary-side positions)
        posP = const.tile([2 * SI, NIB], f32)
        for c in range(2):
            nc.sync.dma_start(
                out=posP[SI * c : SI * (c + 1), :],
                in_=state[:, c : c + 1].rearrange("(b i) x -> i (b x)", b=NIB),
            )

        # --- UXR build (streamed rhs of the pairwise matmul), f32r ---
        #   rows 16*c+ii (0..31): delta(ii'==ii) * pos[SJ*jb+jj, c]  per jb
        #   rows 32+t   (32..47): delta(ii'==t)                      (jb-invariant)
        # Built entirely on-chip (no DMA flood): a K=2 matmul broadcasts the
        # two position rows onto 32 partitions, a select-matmul expands the
        # 16-row delta to 32, and one broadcast-multiply writes rows 0..31.
        uxr = const.tile([48, NJB * 512], f32r)
        # jb-invariant delta block
        d16 = const.tile([SI, SI * SJ], f32)
        nc.gpsimd.memset(d16, 0.0)
        nc.gpsimd.affine_select(
            out=d16.rearrange("p (i j) -> p i j", i=SI),
            in_=d16.rearrange("p (i j) -> p i j", i=SI),
            compare_op=ALU.not_equal,
            fill=1.0,
            base=0,
            channel_multiplier=1,
            pattern=[[-1, SI], [0, SJ]],
        )
        # sel2[k, 16c+ii] = delta(k == ii): doubles d16 to 32 partitions
        sel2 = const.tile([SI, 2 * SI], f32)
        nc.gpsimd.memset(sel2, 0.0)
        nc.gpsimd.affine_select(
            out=sel2.rearrange("p (c i) -> p c i", c=2),
            in_=sel2.rearrange("p (c i) -> p c i", c=2),
            compare_op=ALU.not_equal,
            fill=1.0,
            base=0,
            channel_multiplier=1,
            pattern=[[0, 2], [-1, SI]],
        )
        # cpat[cp, 16c+ii] = delta(cp == c): K=2 position broadcast
        cpat = const.tile([2, 2 * SI], f32)
        nc.gpsimd.memset(cpat, 0.0)
        nc.gpsimd.affine_select(
            out=cpat.rearrange("p (c i) -> p c i", c=2),
            in_=cpat.rearrange("p (c i) -> p c i", c=2),
            compare_op=ALU.not_equal,
            fill=1.0,
            base=0,
            channel_multiplier=1,
            pattern=[[-1, 2], [0, SI]],
        )
        d32_ps = apool.tile([H, 2 * SI * SJ], f32, tag="apsum")
        nc.tensor.matmul(out=d32_ps[0 : 2 * SI, 0 : SI * SJ], lhsT=sel2, rhs=d16, start=True, stop=True)
        d32 = const.tile([2 * SI, SI * SJ], f32)
        nc.vector.tensor_copy(out=d32, in_=d32_ps[0 : 2 * SI, 0 : SI * SJ])
        pp_ps = mpool.tile([H, N], f32, tag="mps")
        nc.tensor.matmul(out=pp_ps[0 : 2 * SI, :], lhsT=cpat, rhs=posT, start=True, stop=True)
        pp_sb = const.tile([2 * SI, N], f32)
        nc.vector.tensor_copy(out=pp_sb, in_=pp_ps[0 : 2 * SI, :])

        def uxr_pos_half(half):
            # rows 0..31, jb-half: pos[j, c] * delta(ii'==ii)
            nb = NJB // 2
            nc.vector.tensor_mul(
                uxr[0 : 2 * SI, 2048 * half : 2048 * (half + 1)]
                .rearrange("p (jb i j) -> p jb i j", i=SI, j=SJ),
                pp_sb[:, 128 * half : 128 * (half + 1)]
                .rearrange("p (jb j) -> p jb j", jb=nb)
                .unsqueeze(2)
                .to_broadcast((2 * SI, nb, SI, SJ)),
                d32.rearrange("p (i j) -> p i j", i=SI)
                .unsqueeze(1)
                .to_broadcast((2 * SI, nb, SI, SJ)),
            )
        # rows 32..47: jb-invariant delta broadcast
        for half in range(2):
            eng = nc.vector.tensor_copy if half == 0 else (lambda out, in_: nc.scalar.copy(out=out, in_=in_))
            eng(
                out=uxr[32:48, 2048 * half : 2048 * (half + 1)]
                .rearrange("p (jb x) -> p jb x", jb=NJB // 2),
                in_=d16.unsqueeze(1).to_broadcast((SI, NJB // 2, SI * SJ)),
            )

        # delta-V pattern (f32r), replicated at the four 32-aligned bases:
        # dv[p, (ii,jj)] = delta(jj == p % 32)
        dv_st = const.tile([128, SI * SJ], f32)
        nc.gpsimd.memset(dv_st, 0.0)
        for g in range(4):
            nc.gpsimd.affine_select(
                out=dv_st[32 * g : 32 * (g + 1), :].rearrange("p (i j) -> p i j", i=SI),
                in_=dv_st[32 * g : 32 * (g + 1), :].rearrange("p (i j) -> p i j", i=SI),
                compare_op=ALU.not_equal,
                fill=1.0,
                base=0,
                channel_multiplier=1,
                pattern=[[0, SI], [-1, SJ]],
            )
        dv = const.tile([128, SI * SJ], f32r)
        nc.scalar.copy(out=dv, in_=dv_st)

        # row-selector for UX rows 32..47: dsel[k, 48v+m] = delta(m>=32, k==16v+m-32)
        dsel = const.tile([128, 8 * 48], f32)
        nc.gpsimd.memset(dsel, 0.0)
        for v in range(8):
            nc.gpsimd.affine_select(
                out=dsel[:, 48 * v + 32 : 48 * v + 48],
                in_=dsel[:, 48 * v + 32 : 48 * v + 48],
                compare_op=ALU.not_equal,
                fill=1.0,
                base=-16 * v,
                channel_multiplier=1,
                pattern=[[-1, 16]],
            )


        # ================= pipelined layer machinery =================
        P = [None] * (L + 1)

        def alloc_ptiles():
            return {
                "up": work.tile([H, N], f32, name="up", tag="up"),
                "vp": work.tile([H, N], f32r, name="vp", tag="vp"),
                "ux": work.tile([48, NIB * H], f32r, name="ux", tag="ux"),
                "dsiln": scratch.tile([H, N], f32, name="dsiln", tag="dsiln"),
            }

        def prep_uv(l, hTl, half, on_act=False):
            """U'/V' for node half `half` of layer l (+ UX position rows).
            on_act routes the elementwise work to ScalarE (startup only,
            while the silu stream hasn't begun)."""
            t = W[l]
            p = P[l]
            sl = slice(H * half, H * (half + 1))
            cp = nc.scalar.copy if on_act else (lambda out, in_: nc.vector.tensor_copy(out=out, in_=in_))
            # U'/V' (node-partition layout): u_ps = (h W1a)-half + sq*r
            u_ps = mpool.tile([H, H], f32, tag="mps")
            nc.tensor.matmul(out=u_ps, lhsT=hTl[:, sl], rhs=t["w1a"], start=True, stop=False)
            nc.tensor.matmul(out=u_ps, lhsT=sq_flat[:, sl], rhs=t["r_sb"], start=False, stop=True)
            cp(out=p["up"][:, sl], in_=u_ps)
            v_ps = mpool.tile([H, H], f32, tag="mps")
            nc.tensor.matmul(out=v_ps, lhsT=hTl[:, sl], rhs=t["w1b"], start=True, stop=False)
            nc.tensor.matmul(out=v_ps, lhsT=sq_flat[:, sl], rhs=t["r_sb"], start=False, stop=True)
            cp(out=p["vp"][:, sl], in_=v_ps)
            # UX rows 0..31 (pos x -2r)
            for v in range(8):
                ib = 8 * half + v
                if False:
                    nc.scalar.activation(
                        out=p["ux"][0 : 2 * SI, H * ib : H * (ib + 1)],
                        in_=t["rneg2b32"],
                        func=AF.Identity,
                        scale=posP[:, ib : ib + 1],
                    )
                else:
                    nc.vector.tensor_scalar_mul(
                        p["ux"][0 : 2 * SI, H * ib : H * (ib + 1)],
                        t["rneg2b32"],
                        posP[:, ib : ib + 1],
                    )

        def prep_ux(l, half, on_act=False, vs=(0, 8)):
            """UX rows 32..47 (U' row-select) for v-range vs via PE + copy."""
            p = P[l]
            sl = slice(H * half, H * (half + 1))
            nv = vs[1] - vs[0]
            ux_ps = apool.tile([48, 8 * H], f32, tag="apsum")
            for k in range(nv):
                v = vs[0] + k
                nc.tensor.matmul(
                    out=ux_ps[:, H * k : H * (k + 1)],
                    lhsT=dsel[:, 48 * v : 48 * (v + 1)],
                    rhs=p["up"][:, sl],
                    start=True,
                    stop=True,
                )
            cp = nc.scalar.copy if on_act else (lambda out, in_: nc.vector.tensor_copy(out=out, in_=in_))
            cp(
                out=p["ux"][
                    2 * SI : 3 * SI,
                    8 * H * half + H * vs[0] : 8 * H * half + H * vs[1],
                ],
                in_=ux_ps[2 * SI : 3 * SI, 0 : H * nv],
            )

        def update_cols(l, hTl, hTn, s_sb, c0, c1):
            """s -= diag; agg = w2^T s + (N-1)b2; h' = h + [h, agg] wu + bu,
            restricted to node columns [c0, c1)."""
            t = W[l]
            p = P[l]
            sl = slice(c0, c1)
            nw = c1 - c0
            # agg = w2^T (s - dsil): accumulate the (tree-independent) diagonal
            # correction first so only the final matmul waits on the reduction
            agg_ps = mpool.tile([H, H], f32, tag="mps")
            nc.tensor.matmul(out=agg_ps[:, 0:nw], lhsT=t["w2_sb"], rhs=p["dsiln"][:, sl], start=True, stop=False)
            nc.tensor.matmul(out=agg_ps[:, 0:nw], lhsT=t["w2_sb"], rhs=s_sb[:, sl], start=False, stop=True)
            agg_sb = scratch.tile([H, H], f32, tag="agg_sb")
            nc.vector.tensor_scalar_add(agg_sb[:, 0:nw], agg_ps[:, 0:nw], t["b2x"])
            upd_ps = mpool.tile([H, H], f32, tag="mps")
            nc.tensor.matmul(out=upd_ps[:, 0:nw], lhsT=t["wu_t"], rhs=hTl[:, sl], start=True, stop=False)
            nc.tensor.matmul(out=upd_ps[:, 0:nw], lhsT=t["wu_b"], rhs=agg_sb[:, 0:nw], start=False, stop=True)
            # h' = (upd + bu) + h   in one DVE op
            nc.vector.scalar_tensor_tensor(
                out=hTn[:, sl],
                in0=upd_ps[:, 0:nw],
                scalar=t["bu_sb"],
                in1=hTl[:, sl],
                op0=ALU.add,
                op1=ALU.add,
            )

        def update_half(l, hTl, hTn, s_sb, half):
            update_cols(l, hTl, hTn, s_sb, H * half, H * (half + 1))

        def pairwise_jhalf(l, ib, jh, sil, t1):
            """Pairwise pre-silu + silu for i-block ib, j-half jh (128 j's)."""
            t = W[l]
            p = P[l]
            for jbp in (2 * jh, 2 * jh + 1):
                aps = apool.tile([H, 2 * SI * SJ], f32, tag="apsum")
                for s in range(2):
                    jb = 2 * jbp + s
                    q = jb % 4
                    nc.tensor.matmul(
                        out=aps[:, 512 * s : 512 * (s + 1)],
                        lhsT=p["ux"][:, H * ib : H * (ib + 1)],
                        rhs=uxr[:, 512 * jb : 512 * (jb + 1)],
                        start=True,
                        stop=False,
                    )
                    nc.tensor.matmul(
                        out=aps[:, 512 * s : 512 * (s + 1)],
                        lhsT=p["vp"][32 * q : 32 * (q + 1), H * (jb // 4) : H * (jb // 4 + 1)],
                        rhs=dv[32 * q : 32 * (q + 1), :],
                        start=False,
                        stop=True,
                        tile_position=(32 * q, 0),
                    )
                # silu(a + b1) into the i-block bf16 buffer
                nc.scalar.activation(
                    out=sil[:, :, 64 * jbp : 64 * (jbp + 1)].rearrange(
                        "p i (s j) -> p i s j", s=2
                    ),
                    in_=aps.rearrange("p (s i j) -> p i s j", s=2, i=SI),
                    func=SILU,
                    bias=t["b1_sb"],
                )
            # bf16 add-tree level 1 (2x DVE mode) over this j-half
            with nc.allow_low_precision(reason="bf16 pairwise sum tree"):
                nc.vector.tensor_add(
                    t1[:, :, 64 * jh : 64 * (jh + 1)],
                    sil[:, :, 128 * jh : 128 * jh + 64],
                    sil[:, :, 128 * jh + 64 : 128 * jh + 128],
                )

        def pairwise_full(l, ib, sil, t1):
            """Whole i-block via 1536/1536/1024-col PSUM tiles (fewer, larger
            silu instructions than the jhalf path)."""
            t = W[l]
            p = P[l]
            for jbs in ((0, 1, 2), (3, 4, 5), (6, 7)):
                nch = len(jbs)
                aps = apool.tile([H, nch * SI * SJ], f32, tag="apsum", name="aps")
                for k, jb in enumerate(jbs):
                    q = jb % 4
                    nc.tensor.matmul(
                        out=aps[:, 512 * k : 512 * (k + 1)],
                        lhsT=p["ux"][:, H * ib : H * (ib + 1)],
                        rhs=uxr[:, 512 * jb : 512 * (jb + 1)],
                        start=True,
                        stop=False,
                    )
                    nc.tensor.matmul(
                        out=aps[:, 512 * k : 512 * (k + 1)],
                        lhsT=p["vp"][32 * q : 32 * (q + 1), H * (jb // 4) : H * (jb // 4 + 1)],
                        rhs=dv[32 * q : 32 * (q + 1), :],
                        start=False,
                        stop=True,
                        tile_position=(32 * q, 0),
                    )
                j0 = 32 * jbs[0]
                nc.scalar.activation(
                    out=sil[:, :, j0 : j0 + 32 * nch].rearrange(
                        "p i (s j) -> p i s j", s=nch
                    ),
                    in_=aps.rearrange("p (s i j) -> p i s j", s=nch, i=SI),
                    func=SILU,
                    bias=t["b1_sb"],
                )
                if jbs[0] == 3:
                    with nc.allow_low_precision(reason="bf16 pairwise sum tree"):
                        nc.vector.tensor_add(
                            t1[:, :, 0:64], sil[:, :, 0:64], sil[:, :, 64:128]
                        )
            with nc.allow_low_precision(reason="bf16 pairwise sum tree"):
                nc.vector.tensor_add(
                    t1[:, :, 64:128], sil[:, :, 128:192], sil[:, :, 192:256]
                )

        def gather_diag(l, ib, sil):
            # the diagonal silu(a_ii) values already exist in the pairwise
            # stream at j == SI*ib + i: pick them out with a strided view
            # (negated, for the fused agg correction matmul)
            flat = sil.rearrange("p i j -> p (i j)")
            nc.vector.tensor_scalar_mul(
                P[l]["dsiln"][:, SI * ib : SI * (ib + 1)],
                flat[:, 16 * ib : 16 * ib + 257 * 15 + 1 : 257],
                -1.0,
            )

        def finish_ib_fast(ib, t1, s_sb):
            """Lower-latency finish: reduce each j-half directly (used for
            the final i-block so the epilogue starts sooner)."""
            sA = scratch.tile([H, SI], f32, tag="sA")
            nc.vector.tensor_reduce(
                out=sA, in_=t1[:, :, 0:64], axis=AX.X, op=ALU.add
            )
            sB = scratch.tile([H, SI], f32, tag="sB")
            nc.vector.tensor_reduce(
                out=sB, in_=t1[:, :, 64:128], axis=AX.X, op=ALU.add
            )
            nc.vector.tensor_add(s_sb[:, SI * ib : SI * (ib + 1)], sA, sB)

        def finish_ib(ib, t1, s_sb):
            """Tree levels 2..3 + final fp32 reduce into s_sb."""
            t2 = scratch.tile([H, SI, N // 4], bf16, tag="t2")
            with nc.allow_low_precision(reason="bf16 pairwise sum tree"):
                nc.vector.tensor_add(t2, t1[:, :, 0:64], t1[:, :, 64:128])
            t3 = scratch.tile([H, SI, N // 8], bf16, tag="t3")
            with nc.allow_low_precision(reason="bf16 pairwise sum tree"):
                nc.vector.tensor_add(t3, t2[:, :, 0:32], t2[:, :, 32:64])
            nc.vector.tensor_reduce(
                out=s_sb[:, SI * ib : SI * (ib + 1)],
                in_=t3,
                axis=AX.X,
                op=ALU.add,
            )

        outw_sb = const.tile([H, 2 * C], f32)
        nc.sync.dma_start(out=outw_sb, in_=out_w[:, :])
        outb_sb = const.tile([2 * C, 1], f32)
        nc.sync.dma_start(out=outb_sb, in_=out_b[:].rearrange("(c x) -> c x", x=1))

        def out_cols(hT4, c0, c1):
            sl = slice(c0, c1)
            nw = c1 - c0
            dl_ps = mpool.tile([2 * C, H], f32, tag="mps")
            nc.tensor.matmul(out=dl_ps[:, 0:nw], lhsT=outw_sb, rhs=hT4[:, sl], start=True, stop=True)
            osb = scratch.tile([2 * C, H], f32, tag="osb")
            nc.vector.scalar_tensor_tensor(
                out=osb[:, 0:nw],
                in0=dl_ps[:, 0:nw],
                scalar=outb_sb,
                in1=stateT[:, sl],
                op0=ALU.add,
                op1=ALU.add,
            )
            nc.sync.dma_start(out=out[c0:c1, :].rearrange("n c -> c n"), in_=osb[:, 0:nw])

        def out_half(hT4, half):
            out_cols(hT4, H * half, H * (half + 1))

        # layer-0 prep (both halves; hT fully known after embedding),
        # interleaved with the two UXR position-row halves so the DVE queue
        # unblocks the j-half-1 matmuls as early as possible
        P[0] = alloc_ptiles()
        uxr_pos_half(0)
        prep_uv(0, hT, 0, on_act=True)
        prep_ux(0, 0, on_act=True)
        uxr_pos_half(1)
        prep_uv(0, hT, 1, on_act=True)
        prep_ux(0, 1, on_act=True)
        if dbg:
            nc.sync.dma_start(out=dbg_t["uxr"][:, :], in_=uxr.bitcast(f32))
            nc.sync.dma_start(out=dbg_t["dv"][:, :], in_=dv.bitcast(f32))
            nc.sync.dma_start(out=dbg_t["ux"][:, :], in_=P[0]["ux"].bitcast(f32))
            nc.sync.dma_start(out=dbg_t["up"][:, :], in_=P[0]["up"])
            nc.sync.dma_start(out=dbg_t["vp"][:, :], in_=P[0]["vp"].bitcast(f32))
            nc.sync.dma_start(out=dbg_t["hT0"][:, :], in_=hT)
            nc.sync.dma_start(out=dbg_t["rb32"][:, :], in_=W[0]["rneg2b32"])
            nc.sync.dma_start(out=dbg_t["posP"][:, :], in_=posP)

        # ================= layers (software-pipelined) =================
        # The half-1 update of layer l-1 (and layer l's half-1 prep) is
        # deferred INTO layer l, after two i-blocks' j-half-0 passes that
        # need only half-0 data — so neither PE nor Act ever drains while
        # DVE finishes the previous layer's reduction tail.
        hT_prev = None   # hT entering layer l-1
        hT_cur = hT      # hT entering layer l
        s_prev = None    # s of layer l-1
        for l in range(L):
            if l + 1 < L:
                W[l + 1] = load_weights(l + 1)
                P[l + 1] = alloc_ptiles()
            s_sb = work.tile([H, N], f32, tag="s_sb")
            hT_new = work.tile([H, N], f32, tag="hT")

            NLEAD = 6
            sils = {}
            for ib in range(NLEAD):
                sil = silup.tile([H, SI, N], bf16, tag="sil")
                t1 = scratch.tile([H, SI, N // 2], bf16, tag="t1", bufs=7)
                sils[ib] = (sil, t1)
                pairwise_jhalf(l, ib, 0, sil, t1)
            if l > 0:
                # finish the previous layer's half-1 update; produce this
                # layer's half-1 V'/UX while PE/Act chew on the j-half-0 work
                update_half(l - 1, hT_prev, hT_cur, s_prev, 1)
                if dbg and l == 1:
                    nc.sync.dma_start(out=dbg_t["s"][:, :], in_=s_prev)
                prep_uv(l, hT_cur, 1)
            for ib in range(NLEAD):
                sil, t1 = sils[ib]
                pairwise_jhalf(l, ib, 1, sil, t1)
                gather_diag(l, ib, sil)
                finish_ib(ib, t1, s_sb)
                if ib == 0 and l > 0:
                    prep_ux(l, 1, vs=(0, 4))
                if ib == 1 and l > 0:
                    prep_ux(l, 1, vs=(4, 8))
            for ib in range(NLEAD, NIB):
                sil = silup.tile([H, SI, N], bf16, tag="sil")
                t1 = scratch.tile([H, SI, N // 2], bf16, tag="t1", bufs=7)
                pairwise_jhalf(l, ib, 0, sil, t1)
                pairwise_jhalf(l, ib, 1, sil, t1)
                gather_diag(l, ib, sil)
                if l == L - 1 and ib == NIB - 1:
                    finish_ib_fast(ib, t1, s_sb)
                else:
                    finish_ib(ib, t1, s_sb)
                if ib == 8:
                    update_half(l, hT_cur, hT_new, s_sb, 0)
                if ib == 9:
                    if l + 1 < L:
                        prep_uv(l + 1, hT_new, 0)
                    else:
                        out_half(hT_new, 0)
                if ib == 10 and l + 1 < L:
                    prep_ux(l + 1, 0, vs=(0, 4))
                if ib == 11 and l + 1 < L:
                    prep_ux(l + 1, 0, vs=(4, 8))
                if l == L - 1 and ib == 13:
                    # third quarter of the final update/output as soon as
                    # i-blocks 8..11 are reduced
                    update_cols(l, hT_cur, hT_new, s_sb, 128, 192)
                if l == L - 1 and ib == 14:
                    out_cols(hT_new, 128, 192)
            hT_prev, hT_cur, s_prev = hT_cur, hT_new, s_sb

        # epilogue: only the final quarter remains
        update_cols(L - 1, hT_prev, hT_cur, s_prev, 192, 256)
        out_cols(hT_cur, 192, 256)

    nc.finalize()
    return nc


def _get_prog(act_name="Silu", dbg=False):
    key = (act_name, dbg)
    if key not in _PROG:
        _PROG[key] = _build_bass(act_name, dbg)
    return _PROG[key]


def run(trace=False, act_name="Silu", **inputs):
    from concourse.bass_utils import run_bass_kernel_spmd

    nc = _get_prog(act_name)
    state = np.ascontiguousarray(np.asarray(inputs["state"], dtype=np.float32))
    shared = {
        k: np.ascontiguousarray(np.asarray(v, dtype=np.float32))
        for k, v in inputs.items()
        if k != "state"
    }
    in_maps = [dict(shared, state=np.ascontiguousarray(state[i])) for i in range(NCORES)]
    res = run_bass_kernel_spmd(nc, in_maps, core_ids=list(range(NCORES)), trace=trace)
    full = np.stack([r["out"] for r in res.results], axis=0)
    return full, res


def kernel(**inputs):
    full, _ = run(trace=False, **inputs)
    return full



# revision 4
# speedup vs baseline: 1.0324x; 1.0324x over previous
"""Trainium2 Bass kernel for nn_NequIPNBodyNetSimple.

Reference computation (per batch b, N=256 particles, H=128 hidden, L=4 layers):
    pos = state[..., :2];  h = state @ embed_w + embed_b
    dist2[i,j] = |pos_i - pos_j|^2
    for l:  m_ij = silu([h_i, h_j, dist2_ij] @ w1 + b1) @ w2 + b2
            agg_i = sum_{j != i} m_ij
            h += concat([h, agg]) @ wu + bu
    out = state + h @ out_w + out_b

Algebraic restructure (exact up to fp reassociation):
    pre-silu a_ij = U'_i + V'_j - 2<p_i, p_j> * r + b1
      with U' = h @ W1a + |p|^2 r,  V' = h @ W1b + |p|^2 r,
           W1a = w1[:H], W1b = w1[H:2H], r = w1[2H]
    agg_i = w2.T @ (sum_{j != i} silu(a_ij)) + (N-1) b2      (linearity of sum)

Sharding: data-parallel over batch B=8 across the 8 NeuronCores (one batch
element per core); parameters replicated.

Performance structure:
  - PE emits the pairwise pre-silu tensor in [128h x 512] chunks as two
    accumulated float32r matmuls (4x the fp32 rate at >=256 moving columns).
  - ScalarE applies silu from PSUM into bf16 SBUF tiles (b1 is folded into
    U' via a rank-1 ones x b1-row matmul, so acts need no bias operand).
  - The final 1024 pre-activations of every other i-block run on the
    Vector engine instead, via a custom 8-stage DVE op computing
    silu(x) ~= relu(x) - |x|*relu(C2 - C1*|x|) — this offloads ~9% of the
    activation columns from ScalarE, balancing it against PE.
  - DVE also does the j-reduction as a bf16 add-tree (2x mode) + final
    fp32 reduce, plus all bias adds (fused scalar_tensor_tensor ops).
  - The layer update (diag-subtract, w2-agg, node update) and the next
    layer's U'/V'/UX prep run PER NODE-HALF, software-pipelined into the
    i-block stream so layer boundaries don't drain the engines.
"""

import numpy as np

B, N, C, H, L = 8, 256, 2, 128, 4
NCORES = 8
SI = 16   # i's per i-block
SJ = 32   # j's per j-block
NIB = N // SI   # 16 i-blocks
NJB = N // SJ   # 8 j-blocks

_PROG = {}
_SILU_OP = [None]
SILU_C1 = 0.069436
SILU_C2 = 0.277212


def _get_silu_op():
    """Register (once) a custom DVE op: out = relu(x) - |x|*relu(C2 - C1*|x|),
    an 8-stage silu approximation used to offload part of the activation
    work from ScalarE to the Vector engine."""
    if _SILU_OP[0] is not None:
        return _SILU_OP[0]
    import concourse.dve_ops as dve_ops
    from concourse.dve_spec import Spec, Src0, C1, C2, Zero, relu, maxx, lower
    from concourse.dve_uop import DveOpSpec

    name = "SILU_APX_NEQ"
    for o in dve_ops.OPS:
        if o.name == name:
            _SILU_OP[0] = o
            return o
    r = relu(Src0)
    t = maxx(Src0, Zero - Src0)
    spec = Spec(
        body=r - t * relu(C2 - t * C1),
        reference=lambda in0, in1, s0, s1, imm2: (
            np.maximum(in0, 0) - np.abs(in0) * np.maximum(imm2 - s1 * np.abs(in0), 0)
        ).astype(np.float32),
    )
    row = max(dve_ops._SUB_OPCODE_FOR_NAME.values()) + 1
    assert row < 0x20
    dve_ops._SUB_OPCODE_FOR_NAME[name] = row
    shas = {}
    for ver in ("v3", "v4"):
        uops = lower(spec, ver=ver)
        shas[ver] = DveOpSpec(name=name, opcode=row, uops=uops, rd1_en=False).sha(ver)
    op = dve_ops.DveOp(name, spec, subdim=False, uops_sha=shas)
    dve_ops.OPS.append(op)
    dve_ops.CUSTOM_DVE_SPECS[name] = spec
    _SILU_OP[0] = op
    return op


def _build_bass(act_name="Silu", dbg=False):
    import concourse.bass as bass
    import concourse.mybir as mybir
    import concourse.tile as tile
    from concourse import bacc
    from contextlib import ExitStack

    f32 = mybir.dt.float32
    f32r = mybir.dt.float32r
    bf16 = mybir.dt.bfloat16
    AF = mybir.ActivationFunctionType
    ALU = mybir.AluOpType
    AX = mybir.AxisListType
    SILU = getattr(AF, act_name)

    nc = bacc.Bacc()

    # ---- DRAM I/O (names must match setup_inputs keys) ----
    state = nc.dram_tensor("state", [N, 2 * C], f32, kind="ExternalInput")
    embed_w = nc.dram_tensor("embed_w", [2 * C, H], f32, kind="ExternalInput")
    embed_b = nc.dram_tensor("embed_b", [H], f32, kind="ExternalInput")
    w1 = nc.dram_tensor("w1", [L, 2 * H + 1, H], f32, kind="ExternalInput")
    b1 = nc.dram_tensor("b1", [L, H], f32, kind="ExternalInput")
    w2 = nc.dram_tensor("w2", [L, H, H], f32, kind="ExternalInput")
    b2 = nc.dram_tensor("b2", [L, H], f32, kind="ExternalInput")
    wu = nc.dram_tensor("wu", [L, 2 * H, H], f32, kind="ExternalInput")
    bu = nc.dram_tensor("bu", [L, H], f32, kind="ExternalInput")
    out_w = nc.dram_tensor("out_w", [H, 2 * C], f32, kind="ExternalInput")
    out_b = nc.dram_tensor("out_b", [2 * C], f32, kind="ExternalInput")
    out = nc.dram_tensor("out", [N, 2 * C], f32, kind="ExternalOutput")
    if dbg:
        dbg_t = {
            nm: nc.dram_tensor("dbg_" + nm, shp, f32, kind="ExternalOutput")
            for nm, shp in [
                ("uxr", [48, 8 * 512]), ("dv", [128, 512]), ("ux", [48, 16 * 128]),
                ("up", [H, N]), ("vp", [H, N]), ("s", [H, N]), ("hT0", [H, N]),
                ("rb32", [32, H]), ("posP", [32, 16]),
            ]
        }

    with tile.TileContext(nc) as tc, ExitStack() as ctx:
        const = ctx.enter_context(tc.tile_pool(name="const", bufs=1))
        wpool = ctx.enter_context(tc.tile_pool(name="wpool", bufs=2))
        work = ctx.enter_context(tc.tile_pool(name="work", bufs=2))
        silup = ctx.enter_context(tc.tile_pool(name="silup", bufs=7))
        scratch = ctx.enter_context(tc.tile_pool(name="scratch", bufs=2))
        apool = ctx.enter_context(tc.tile_pool(name="apool", bufs=3, space="PSUM"))
        mpool = ctx.enter_context(tc.tile_pool(name="mpool", bufs=2, space="PSUM"))

        # ================= one-time setup =================
        # ones132: broadcasts a [1, H] row onto 32 partitions via matmul
        ones132 = const.tile([1, 2 * SI], f32)
        nc.vector.memset(ones132, 1.0)
        ones128 = const.tile([1, H], f32)
        nc.vector.memset(ones128, 1.0)
        # PE p-state warm-up: keep the tensor engine busy while the input
        # DMAs land so the first real matmuls run at full clock
        warm = const.tile([1, N], f32)
        nc.vector.memset(warm, 0.0)
        warm_ps = apool.tile([H, 2 * SI * SJ], f32, tag="apsum")
        for w in range(2):
            nc.tensor.matmul(
                out=warm_ps[0 : 2 * SI, 0:N],
                lhsT=ones132,
                rhs=warm,
                start=True,
                stop=True,
            )

        def load_weights(l):
            t = {}
            t["w1a"] = wpool.tile([H, H], f32, name="w1a", tag="w1a")
            nc.sync.dma_start(out=t["w1a"], in_=w1[l, 0:H, :])
            t["w1b"] = wpool.tile([H, H], f32, name="w1b", tag="w1b")
            nc.sync.dma_start(out=t["w1b"], in_=w1[l, H : 2 * H, :])
            t["r_sb"] = wpool.tile([1, H], f32, name="r_sb", tag="r_sb")
            nc.sync.dma_start(out=t["r_sb"], in_=w1[l, 2 * H : 2 * H + 1, :])
            t["b1row"] = wpool.tile([1, H], f32, name="b1row", tag="b1row")
            nc.sync.dma_start(out=t["b1row"], in_=b1[l : l + 1, :])
            t["w2_sb"] = wpool.tile([H, H], f32, name="w2_sb", tag="w2_sb")
            nc.sync.dma_start(out=t["w2_sb"], in_=w2[l, :, :])
            t["wu_t"] = wpool.tile([H, H], f32, name="wu_t", tag="wu_t")
            nc.sync.dma_start(out=t["wu_t"], in_=wu[l, 0:H, :])
            t["wu_b"] = wpool.tile([H, H], f32, name="wu_b", tag="wu_b")
            nc.sync.dma_start(out=t["wu_b"], in_=wu[l, H : 2 * H, :])
            t["b2_sb"] = wpool.tile([H, 1], f32, name="b2_sb", tag="b2_sb")
            nc.sync.dma_start(out=t["b2_sb"], in_=b2[l, :].rearrange("(h x) -> h x", x=1))
            t["bu_sb"] = wpool.tile([H, 1], f32, name="bu_sb", tag="bu_sb")
            nc.sync.dma_start(out=t["bu_sb"], in_=bu[l, :].rearrange("(h x) -> h x", x=1))
            t["b2x"] = wpool.tile([H, 1], f32, name="b2x", tag="b2x")
            nc.vector.tensor_scalar_mul(t["b2x"], t["b2_sb"], float(N - 1))
            t["rneg2"] = wpool.tile([1, H], f32, name="rneg2", tag="rneg2")
            nc.vector.tensor_scalar_mul(t["rneg2"], t["r_sb"], -2.0)
            # rneg2b32[p, h] = -2*r[h] on 32 partitions (for the UX pos rows)
            r32_ps = mpool.tile([H, N], f32, tag="mps")
            nc.tensor.matmul(
                out=r32_ps[0 : 2 * SI, 0:H], lhsT=ones132, rhs=t["rneg2"],
                start=True, stop=True,
            )
            t["rneg2b32"] = wpool.tile([2 * SI, H], f32, name="rneg2b32", tag="rneg2b32")
            nc.vector.tensor_copy(out=t["rneg2b32"], in_=r32_ps[0 : 2 * SI, 0:H])
            return t

        # stateT[c, i] = state[i, c]   (4 x 256)
        stateT = const.tile([2 * C, N], f32)
        nc.sync.dma_start(out=stateT, in_=state[:, :].rearrange("n c -> c n"))
        posT = stateT[0:2, :]  # [2, 256]

        # sq_flat[0, i] = |pos_i|^2  via ones-matmul partition reduction
        possq = const.tile([2, N], f32)
        nc.vector.tensor_mul(possq, posT, posT)
        ones2 = const.tile([2, 1], f32)
        nc.vector.memset(ones2, 1.0)
        sq_ps = mpool.tile([H, N], f32, tag="mps")
        nc.tensor.matmul(out=sq_ps[0:1, :], lhsT=ones2, rhs=possq, start=True, stop=True)
        sq_flat = const.tile([1, N], f32)
        nc.vector.tensor_copy(out=sq_flat, in_=sq_ps[0:1, :])

        # ---- embedding: hT = (state @ embed_w + embed_b)^T ----
        embw_sb = const.tile([2 * C, H], f32)
        nc.sync.dma_start(out=embw_sb, in_=embed_w[:, :])
        embb_sb = const.tile([H, 1], f32)
        nc.sync.dma_start(out=embb_sb, in_=embed_b[:].rearrange("(h x) -> h x", x=1))
        h_ps = mpool.tile([H, N], f32, tag="mps")
        nc.tensor.matmul(out=h_ps, lhsT=embw_sb, rhs=stateT, start=True, stop=True)
        hT = work.tile([H, N], f32, tag="hT")
        nc.vector.tensor_scalar_add(hT, h_ps, embb_sb)

        W = [None] * L
        W[0] = load_weights(0)

        # posP[16*c+ii, ib] = pos[SI*ib+ii, c]  (stationary-side positions)
        posP = const.tile([2 * SI, NIB], f32)
        for c in range(2):
            nc.sync.dma_start(
                out=posP[SI * c : SI * (c + 1), :],
                in_=state[:, c : c + 1].rearrange("(b i) x -> i (b x)", b=NIB),
            )

        # --- UXR build (streamed rhs of the pairwise matmul), f32r ---
        #   rows 16*c+ii (0..31): delta(ii'==ii) * pos[SJ*jb+jj, c]  per jb
        #   rows 32+t   (32..47): delta(ii'==t)                      (jb-invariant)
        # Built entirely on-chip (no DMA flood): a K=2 matmul broadcasts the
        # two position rows onto 32 partitions, a select-matmul expands the
        # 16-row delta to 32, and one broadcast-multiply writes rows 0..31.
        uxr = const.tile([48, NJB * 512], f32r)
        # jb-invariant delta block
        d16 = const.tile([SI, SI * SJ], f32)
        nc.gpsimd.memset(d16, 0.0)
        nc.gpsimd.affine_select(
            out=d16.rearrange("p (i j) -> p i j", i=SI),
            in_=d16.rearrange("p (i j) -> p i j", i=SI),
            compare_op=ALU.not_equal,
            fill=1.0,
            base=0,
            channel_multiplier=1,
            pattern=[[-1, SI], [0, SJ]],
        )
        # sel2[k, 16c+ii] = delta(k == ii): doubles d16 to 32 partitions
        sel2 = const.tile([SI, 2 * SI], f32)
        nc.gpsimd.memset(sel2, 0.0)
        nc.gpsimd.affine_select(
            out=sel2.rearrange("p (c i) -> p c i", c=2),
            in_=sel2.rearrange("p (c i) -> p c i", c=2),
            compare_op=ALU.not_equal,
            fill=1.0,
            base=0,
            channel_multiplier=1,
            pattern=[[0, 2], [-1, SI]],
        )
        # cpat[cp, 16c+ii] = delta(cp == c): K=2 position broadcast
        cpat = const.tile([2, 2 * SI], f32)
        nc.gpsimd.memset(cpat, 0.0)
        nc.gpsimd.affine_select(
            out=cpat.rearrange("p (c i) -> p c i", c=2),
            in_=cpat.rearrange("p (c i) -> p c i", c=2),
            compare_op=ALU.not_equal,
            fill=1.0,
            base=0,
            channel_multiplier=1,
            pattern=[[-1, 2], [0, SI]],
        )
        d32_ps = apool.tile([H, 2 * SI * SJ], f32, tag="apsum")
        nc.tensor.matmul(out=d32_ps[0 : 2 * SI, 0 : SI * SJ], lhsT=sel2, rhs=d16, start=True, stop=True)
        d32 = const.tile([2 * SI, SI * SJ], f32)
        nc.vector.tensor_copy(out=d32, in_=d32_ps[0 : 2 * SI, 0 : SI * SJ])
        pp_ps = mpool.tile([H, N], f32, tag="mps")
        nc.tensor.matmul(out=pp_ps[0 : 2 * SI, :], lhsT=cpat, rhs=posT, start=True, stop=True)
        pp_sb = const.tile([2 * SI, N], f32)
        nc.vector.tensor_copy(out=pp_sb, in_=pp_ps[0 : 2 * SI, :])

        def uxr_pos_half(half):
            # rows 0..31, jb-half: pos[j, c] * delta(ii'==ii)
            nb = NJB // 2
            nc.vector.tensor_mul(
                uxr[0 : 2 * SI, 2048 * half : 2048 * (half + 1)]
                .rearrange("p (jb i j) -> p jb i j", i=SI, j=SJ),
                pp_sb[:, 128 * half : 128 * (half + 1)]
                .rearrange("p (jb j) -> p jb j", jb=nb)
                .unsqueeze(2)
                .to_broadcast((2 * SI, nb, SI, SJ)),
                d32.rearrange("p (i j) -> p i j", i=SI)
                .unsqueeze(1)
                .to_broadcast((2 * SI, nb, SI, SJ)),
            )
        # rows 32..47: jb-invariant delta broadcast
        for half in range(2):
            eng = nc.vector.tensor_copy if half == 0 else (lambda out, in_: nc.scalar.copy(out=out, in_=in_))
            eng(
                out=uxr[32:48, 2048 * half : 2048 * (half + 1)]
                .rearrange("p (jb x) -> p jb x", jb=NJB // 2),
                in_=d16.unsqueeze(1).to_broadcast((SI, NJB // 2, SI * SJ)),
            )

        # delta-V pattern (f32r), replicated at the four 32-aligned bases:
        # dv[p, (ii,jj)] = delta(jj == p % 32)
        dv_st = const.tile([128, SI * SJ], f32)
        nc.gpsimd.memset(dv_st, 0.0)
        for g in range(4):
            nc.gpsimd.affine_select(
                out=dv_st[32 * g : 32 * (g + 1), :].rearrange("p (i j) -> p i j", i=SI),
                in_=dv_st[32 * g : 32 * (g + 1), :].rearrange("p (i j) -> p i j", i=SI),
                compare_op=ALU.not_equal,
                fill=1.0,
                base=0,
                channel_multiplier=1,
                pattern=[[0, SI], [-1, SJ]],
            )
        dv = const.tile([128, SI * SJ], f32r)
        nc.scalar.copy(out=dv, in_=dv_st)

        # row-selector for UX rows 32..47: dsel[k, 48v+m] = delta(m>=32, k==16v+m-32)
        dsel = const.tile([128, 8 * 48], f32)
        nc.gpsimd.memset(dsel, 0.0)
        for v in range(8):
            nc.gpsimd.affine_select(
                out=dsel[:, 48 * v + 32 : 48 * v + 48],
                in_=dsel[:, 48 * v + 32 : 48 * v + 48],
                compare_op=ALU.not_equal,
                fill=1.0,
                base=-16 * v,
                channel_multiplier=1,
                pattern=[[-1, 16]],
            )


        # ================= pipelined layer machinery =================
        P = [None] * (L + 1)

        def alloc_ptiles():
            return {
                "up": work.tile([H, N], f32, name="up", tag="up"),
                "vp": work.tile([H, N], f32r, name="vp", tag="vp"),
                "ux": work.tile([48, NIB * H], f32r, name="ux", tag="ux"),
                "dsiln": scratch.tile([H, N], f32, name="dsiln", tag="dsiln"),
            }

        def prep_uv(l, hTl, half, on_act=False):
            """U'/V' for node half `half` of layer l (+ UX position rows).
            on_act routes the elementwise work to ScalarE (startup only,
            while the silu stream hasn't begun)."""
            t = W[l]
            p = P[l]
            sl = slice(H * half, H * (half + 1))
            cp = nc.scalar.copy if on_act else (lambda out, in_: nc.vector.tensor_copy(out=out, in_=in_))
            # U'/V' (node-partition layout): u_ps = (h W1a)-half + sq*r
            u_ps = mpool.tile([H, H], f32, tag="mps")
            nc.tensor.matmul(out=u_ps, lhsT=hTl[:, sl], rhs=t["w1a"], start=True, stop=False)
            nc.tensor.matmul(out=u_ps, lhsT=sq_flat[:, sl], rhs=t["r_sb"], start=False, stop=False)
            nc.tensor.matmul(out=u_ps, lhsT=ones128, rhs=t["b1row"], start=False, stop=True)
            cp(out=p["up"][:, sl], in_=u_ps)
            v_ps = mpool.tile([H, H], f32, tag="mps")
            nc.tensor.matmul(out=v_ps, lhsT=hTl[:, sl], rhs=t["w1b"], start=True, stop=False)
            nc.tensor.matmul(out=v_ps, lhsT=sq_flat[:, sl], rhs=t["r_sb"], start=False, stop=True)
            cp(out=p["vp"][:, sl], in_=v_ps)
            # UX rows 0..31 (pos x -2r)
            for v in range(8):
                ib = 8 * half + v
                if False:
                    nc.scalar.activation(
                        out=p["ux"][0 : 2 * SI, H * ib : H * (ib + 1)],
                        in_=t["rneg2b32"],
                        func=AF.Identity,
                        scale=posP[:, ib : ib + 1],
                    )
                else:
                    nc.vector.tensor_scalar_mul(
                        p["ux"][0 : 2 * SI, H * ib : H * (ib + 1)],
                        t["rneg2b32"],
                        posP[:, ib : ib + 1],
                    )

        def prep_ux(l, half, on_act=False, vs=(0, 8)):
            """UX rows 32..47 (U' row-select) for v-range vs via PE + copy."""
            p = P[l]
            sl = slice(H * half, H * (half + 1))
            nv = vs[1] - vs[0]
            ux_ps = apool.tile([48, 8 * H], f32, tag="apsum")
            for k in range(nv):
                v = vs[0] + k
                nc.tensor.matmul(
                    out=ux_ps[:, H * k : H * (k + 1)],
                    lhsT=dsel[:, 48 * v : 48 * (v + 1)],
                    rhs=p["up"][:, sl],
                    start=True,
                    stop=True,
                )
            cp = nc.scalar.copy if on_act else (lambda out, in_: nc.vector.tensor_copy(out=out, in_=in_))
            cp(
                out=p["ux"][
                    2 * SI : 3 * SI,
                    8 * H * half + H * vs[0] : 8 * H * half + H * vs[1],
                ],
                in_=ux_ps[2 * SI : 3 * SI, 0 : H * nv],
            )

        def update_cols(l, hTl, hTn, s_sb, c0, c1):
            """s -= diag; agg = w2^T s + (N-1)b2; h' = h + [h, agg] wu + bu,
            restricted to node columns [c0, c1)."""
            t = W[l]
            p = P[l]
            sl = slice(c0, c1)
            nw = c1 - c0
            # agg = w2^T (s - dsil): accumulate the (tree-independent) diagonal
            # correction first so only the final matmul waits on the reduction
            agg_ps = mpool.tile([H, H], f32, tag="mps")
            nc.tensor.matmul(out=agg_ps[:, 0:nw], lhsT=t["w2_sb"], rhs=p["dsiln"][:, sl], start=True, stop=False)
            nc.tensor.matmul(out=agg_ps[:, 0:nw], lhsT=t["w2_sb"], rhs=s_sb[:, sl], start=False, stop=True)
            agg_sb = scratch.tile([H, H], f32, tag="agg_sb")
            nc.vector.tensor_scalar_add(agg_sb[:, 0:nw], agg_ps[:, 0:nw], t["b2x"])
            upd_ps = mpool.tile([H, H], f32, tag="mps")
            nc.tensor.matmul(out=upd_ps[:, 0:nw], lhsT=t["wu_t"], rhs=hTl[:, sl], start=True, stop=False)
            nc.tensor.matmul(out=upd_ps[:, 0:nw], lhsT=t["wu_b"], rhs=agg_sb[:, 0:nw], start=False, stop=True)
            # h' = (upd + bu) + h   in one DVE op
            nc.vector.scalar_tensor_tensor(
                out=hTn[:, sl],
                in0=upd_ps[:, 0:nw],
                scalar=t["bu_sb"],
                in1=hTl[:, sl],
                op0=ALU.add,
                op1=ALU.add,
            )

        def update_half(l, hTl, hTn, s_sb, half):
            update_cols(l, hTl, hTn, s_sb, H * half, H * (half + 1))

        def pairwise_jhalf(l, ib, jh, sil, t1):
            """Pairwise pre-silu + silu for i-block ib, j-half jh (128 j's)."""
            t = W[l]
            p = P[l]
            for jbp in (2 * jh, 2 * jh + 1):
                aps = apool.tile([H, 2 * SI * SJ], f32, tag="apsum")
                for s in range(2):
                    jb = 2 * jbp + s
                    q = jb % 4
                    nc.tensor.matmul(
                        out=aps[:, 512 * s : 512 * (s + 1)],
                        lhsT=p["ux"][:, H * ib : H * (ib + 1)],
                        rhs=uxr[:, 512 * jb : 512 * (jb + 1)],
                        start=True,
                        stop=False,
                    )
                    nc.tensor.matmul(
                        out=aps[:, 512 * s : 512 * (s + 1)],
                        lhsT=p["vp"][32 * q : 32 * (q + 1), H * (jb // 4) : H * (jb // 4 + 1)],
                        rhs=dv[32 * q : 32 * (q + 1), :],
                        start=False,
                        stop=True,
                        tile_position=(32 * q, 0),
                    )
                # silu into the i-block bf16 buffer (b1 folded into U');
                # the final 1024 of odd i-blocks runs on the Vector engine
                # via the custom approx op, offloading ScalarE
                if jbp == 3 and (ib % 2) == 1 and ib >= 5:
                    with nc.allow_low_precision(reason="silu approx offload"):
                        for s in range(2):
                            nc.vector._custom_dve(
                                silu_op,
                                out=sil[:, :, 64 * jbp + 32 * s : 64 * jbp + 32 * (s + 1)],
                                in0=aps[:, 512 * s : 512 * (s + 1)].rearrange(
                                    "p (x j) -> p x j", j=SJ
                                ),
                                s1=SILU_C1,
                                imm2=SILU_C2,
                            )
                else:
                    nc.scalar.activation(
                        out=sil[:, :, 64 * jbp : 64 * (jbp + 1)].rearrange(
                            "p i (s j) -> p i s j", s=2
                        ),
                        in_=aps.rearrange("p (s i j) -> p i s j", s=2, i=SI),
                        func=SILU,
                    )
            # bf16 add-tree level 1 (2x DVE mode) over this j-half
            with nc.allow_low_precision(reason="bf16 pairwise sum tree"):
                nc.vector.tensor_add(
                    t1[:, :, 64 * jh : 64 * (jh + 1)],
                    sil[:, :, 128 * jh : 128 * jh + 64],
                    sil[:, :, 128 * jh + 64 : 128 * jh + 128],
                )

        silu_op = _get_silu_op()

        def pairwise_full(l, ib, sil, t1):
            """Whole i-block via 1536/1536/1024-col PSUM tiles; for odd
            i-blocks the final (jb 6,7) tile's silu runs on the Vector
            engine via the custom approx op (offloads ScalarE)."""
            t = W[l]
            p = P[l]
            for jbs in ((0, 1, 2), (3, 4, 5), (6, 7)):
                nch = len(jbs)
                aps = apool.tile([H, nch * SI * SJ], f32, tag="apsum", name="aps")
                for k, jb in enumerate(jbs):
                    q = jb % 4
                    nc.tensor.matmul(
                        out=aps[:, 512 * k : 512 * (k + 1)],
                        lhsT=p["ux"][:, H * ib : H * (ib + 1)],
                        rhs=uxr[:, 512 * jb : 512 * (jb + 1)],
                        start=True,
                        stop=False,
                    )
                    nc.tensor.matmul(
                        out=aps[:, 512 * k : 512 * (k + 1)],
                        lhsT=p["vp"][32 * q : 32 * (q + 1), H * (jb // 4) : H * (jb // 4 + 1)],
                        rhs=dv[32 * q : 32 * (q + 1), :],
                        start=False,
                        stop=True,
                        tile_position=(32 * q, 0),
                    )
                j0 = 32 * jbs[0]
                nc.scalar.activation(
                    out=sil[:, :, j0 : j0 + 32 * nch].rearrange(
                        "p i (s j) -> p i s j", s=nch
                    ),
                    in_=aps.rearrange("p (s i j) -> p i s j", s=nch, i=SI),
                    func=SILU,
                )
                if jbs[0] == 3:
                    with nc.allow_low_precision(reason="bf16 pairwise sum tree"):
                        nc.vector.tensor_add(
                            t1[:, :, 0:64], sil[:, :, 0:64], sil[:, :, 64:128]
                        )
            with nc.allow_low_precision(reason="bf16 pairwise sum tree"):
                nc.vector.tensor_add(
                    t1[:, :, 64:128], sil[:, :, 128:192], sil[:, :, 192:256]
                )

        def gather_diag(l, ib, sil):
            # the diagonal silu(a_ii) values already exist in the pairwise
            # stream at j == SI*ib + i: pick them out with a strided view
            # (negated, for the fused agg correction matmul)
            flat = sil.rearrange("p i j -> p (i j)")
            nc.vector.tensor_scalar_mul(
                P[l]["dsiln"][:, SI * ib : SI * (ib + 1)],
                flat[:, 16 * ib : 16 * ib + 257 * 15 + 1 : 257],
                -1.0,
            )

        def finish_ib_fast(ib, t1, s_sb):
            """Lower-latency finish: reduce each j-half directly (used for
            the final i-block so the epilogue starts sooner)."""
            sA = scratch.tile([H, SI], f32, tag="sA")
            nc.vector.tensor_reduce(
                out=sA, in_=t1[:, :, 0:64], axis=AX.X, op=ALU.add
            )
            sB = scratch.tile([H, SI], f32, tag="sB")
            nc.vector.tensor_reduce(
                out=sB, in_=t1[:, :, 64:128], axis=AX.X, op=ALU.add
            )
            nc.vector.tensor_add(s_sb[:, SI * ib : SI * (ib + 1)], sA, sB)

        def finish_ib(ib, t1, s_sb):
            """Tree levels 2..3 + final fp32 reduce into s_sb."""
            t2 = scratch.tile([H, SI, N // 4], bf16, tag="t2")
            with nc.allow_low_precision(reason="bf16 pairwise sum tree"):
                nc.vector.tensor_add(t2, t1[:, :, 0:64], t1[:, :, 64:128])
            t3 = scratch.tile([H, SI, N // 8], bf16, tag="t3")
            with nc.allow_low_precision(reason="bf16 pairwise sum tree"):
                nc.vector.tensor_add(t3, t2[:, :, 0:32], t2[:, :, 32:64])
            nc.vector.tensor_reduce(
                out=s_sb[:, SI * ib : SI * (ib + 1)],
                in_=t3,
                axis=AX.X,
                op=ALU.add,
            )

        outw_sb = const.tile([H, 2 * C], f32)
        nc.sync.dma_start(out=outw_sb, in_=out_w[:, :])
        outb_sb = const.tile([2 * C, 1], f32)
        nc.sync.dma_start(out=outb_sb, in_=out_b[:].rearrange("(c x) -> c x", x=1))

        def out_cols(hT4, c0, c1):
            sl = slice(c0, c1)
            nw = c1 - c0
            dl_ps = mpool.tile([2 * C, H], f32, tag="mps")
            nc.tensor.matmul(out=dl_ps[:, 0:nw], lhsT=outw_sb, rhs=hT4[:, sl], start=True, stop=True)
            osb = scratch.tile([2 * C, H], f32, tag="osb")
            nc.vector.scalar_tensor_tensor(
                out=osb[:, 0:nw],
                in0=dl_ps[:, 0:nw],
                scalar=outb_sb,
                in1=stateT[:, sl],
                op0=ALU.add,
                op1=ALU.add,
            )
            nc.sync.dma_start(out=out[c0:c1, :].rearrange("n c -> c n"), in_=osb[:, 0:nw])

        def out_half(hT4, half):
            out_cols(hT4, H * half, H * (half + 1))

        # layer-0 prep (both halves; hT fully known after embedding),
        # interleaved with the two UXR position-row halves so the DVE queue
        # unblocks the j-half-1 matmuls as early as possible
        P[0] = alloc_ptiles()
        uxr_pos_half(0)
        prep_uv(0, hT, 0, on_act=True)
        prep_ux(0, 0, on_act=True)
        uxr_pos_half(1)
        prep_uv(0, hT, 1, on_act=True)
        prep_ux(0, 1, on_act=True)
        if dbg:
            nc.sync.dma_start(out=dbg_t["uxr"][:, :], in_=uxr.bitcast(f32))
            nc.sync.dma_start(out=dbg_t["dv"][:, :], in_=dv.bitcast(f32))
            nc.sync.dma_start(out=dbg_t["ux"][:, :], in_=P[0]["ux"].bitcast(f32))
            nc.sync.dma_start(out=dbg_t["up"][:, :], in_=P[0]["up"])
            nc.sync.dma_start(out=dbg_t["vp"][:, :], in_=P[0]["vp"].bitcast(f32))
            nc.sync.dma_start(out=dbg_t["hT0"][:, :], in_=hT)
            nc.sync.dma_start(out=dbg_t["rb32"][:, :], in_=W[0]["rneg2b32"])
            nc.sync.dma_start(out=dbg_t["posP"][:, :], in_=posP)

        # ================= layers (software-pipelined) =================
        # The half-1 update of layer l-1 (and layer l's half-1 prep) is
        # deferred INTO layer l, after two i-blocks' j-half-0 passes that
        # need only half-0 data — so neither PE nor Act ever drains while
        # DVE finishes the previous layer's reduction tail.
        hT_prev = None   # hT entering layer l-1
        hT_cur = hT      # hT entering layer l
        s_prev = None    # s of layer l-1
        for l in range(L):
            if l + 1 < L:
                W[l + 1] = load_weights(l + 1)
                P[l + 1] = alloc_ptiles()
            s_sb = work.tile([H, N], f32, tag="s_sb")
            hT_new = work.tile([H, N], f32, tag="hT")

            NLEAD = 6
            sils = {}
            for ib in range(NLEAD):
                sil = silup.tile([H, SI, N], bf16, tag="sil")
                t1 = scratch.tile([H, SI, N // 2], bf16, tag="t1", bufs=7)
                sils[ib] = (sil, t1)
                pairwise_jhalf(l, ib, 0, sil, t1)
            if l > 0:
                # finish the previous layer's half-1 update; produce this
                # layer's half-1 V'/UX while PE/Act chew on the j-half-0 work
                update_half(l - 1, hT_prev, hT_cur, s_prev, 1)
                if dbg and l == 1:
                    nc.sync.dma_start(out=dbg_t["s"][:, :], in_=s_prev)
                prep_uv(l, hT_cur, 1)
            for ib in range(NLEAD):
                sil, t1 = sils[ib]
                pairwise_jhalf(l, ib, 1, sil, t1)
                gather_diag(l, ib, sil)
                finish_ib(ib, t1, s_sb)
                if ib == 0 and l > 0:
                    prep_ux(l, 1, vs=(0, 4))
                if ib == 1 and l > 0:
                    prep_ux(l, 1, vs=(4, 8))
            for ib in range(NLEAD, NIB):
                sil = silup.tile([H, SI, N], bf16, tag="sil")
                t1 = scratch.tile([H, SI, N // 2], bf16, tag="t1", bufs=7)
                pairwise_jhalf(l, ib, 0, sil, t1)
                pairwise_jhalf(l, ib, 1, sil, t1)
                gather_diag(l, ib, sil)
                if l == L - 1 and ib == NIB - 1:
                    finish_ib_fast(ib, t1, s_sb)
                else:
                    finish_ib(ib, t1, s_sb)
                if ib == 8:
                    update_half(l, hT_cur, hT_new, s_sb, 0)
                if ib == 9:
                    if l + 1 < L:
                        prep_uv(l + 1, hT_new, 0)
                    else:
                        out_half(hT_new, 0)
                if ib == 10 and l + 1 < L:
                    prep_ux(l + 1, 0, vs=(0, 4))
                if ib == 11 and l + 1 < L:
                    prep_ux(l + 1, 0, vs=(4, 8))
                if l == L - 1 and ib == 13:
                    # third quarter of the final update/output as soon as
                    # i-blocks 8..11 are reduced
                    update_cols(l, hT_cur, hT_new, s_sb, 128, 192)
                if l == L - 1 and ib == 14:
                    out_cols(hT_new, 128, 192)
            hT_prev, hT_cur, s_prev = hT_cur, hT_new, s_sb

        # epilogue: only the final quarter remains
        update_cols(L - 1, hT_prev, hT_cur, s_prev, 192, 256)
        out_cols(hT_cur, 192, 256)

    nc.finalize()
    return nc


def _get_prog(act_name="Silu", dbg=False):
    key = (act_name, dbg)
    if key not in _PROG:
        _PROG[key] = _build_bass(act_name, dbg)
    return _PROG[key]


def run(trace=False, act_name="Silu", **inputs):
    from concourse.bass_utils import run_bass_kernel_spmd

    nc = _get_prog(act_name)
    state = np.ascontiguousarray(np.asarray(inputs["state"], dtype=np.float32))
    shared = {
        k: np.ascontiguousarray(np.asarray(v, dtype=np.float32))
        for k, v in inputs.items()
        if k != "state"
    }
    in_maps = [dict(shared, state=np.ascontiguousarray(state[i])) for i in range(NCORES)]
    res = run_bass_kernel_spmd(nc, in_maps, core_ids=list(range(NCORES)), trace=trace)
    full = np.stack([r["out"] for r in res.results], axis=0)
    return full, res


def kernel(**inputs):
    full, _ = run(trace=False, **inputs)
    return full

